# revision 1
# baseline (speedup 1.0000x reference)
"""GRU-D decoder kernel for Trainium2 (8 NeuronCores, data-parallel over batch).

Math (mask == ones everywhere, which the reference hardcodes):
  x_hat = C (constant), d = dt broadcast, gamma_x unused.
  gamma[t,b,j] = exp(-relu(dt[t,b] * colsum(Wgh)[j] + bgh[j]))   (precomputed host-side)
  per step: hdec = gamma_t * h
            z = sigmoid(hdec @ Wz_h + Az0);  r = sigmoid(hdec @ Wr_h + Ar0)
            htl = tanh((r*hdec) @ Wh_h + Ah0)
            h = hdec + z*(htl - hdec)
  out[t] = h_t @ Wlin + blin
  where A?0 = C @ W?_x + colsum(W?_m) + b?  (time-constant, precomputed host-side).

Device layout: everything transposed (H on partitions as 4 tiles of 128,
batch=64 on the free dim), packed as SBUF tiles (128, 4*64) with column
index = kt*64 + b.  Gate matmuls use the weight blocks as stationary
operands and hdec slices as moving operands; outputs land natively in the
same transposed layout, so no transposes are needed anywhere.  The
per-step tail (tanh/blend/decay) is split into two column halves so the
tensor engine can start the next group while the tail of the previous
half is still on Scalar/Vector.
"""

import numpy as np
import ml_dtypes

T, B, H, O = 100, 512, 512, 512
NCORES = 8
BL = B // NCORES  # 64
KC = 4  # contraction chunks of 128
JT = 4  # output j-tiles of 128
FR = JT * BL  # 256
HB = FR // 2  # 128 (half of the free dim; = 2 j-tiles)
GCH = 20  # gamma chunk (steps per DMA)

_BUILD_CACHE = {}


def _build_program():
    if "nc" in _BUILD_CACHE:
        return _BUILD_CACHE["nc"]

    import concourse.tile as tile
    import concourse.mybir as mybir
    from concourse import bacc
    from contextlib import ExitStack

    f32 = mybir.dt.float32
    bf16 = mybir.dt.bfloat16
    AF = mybir.ActivationFunctionType

    nc = bacc.Bacc("TRN2", target_bir_lowering=False, debug=False,
                   num_devices=NCORES)

    gam_d = nc.dram_tensor("gam", [128, T, FR], f32, kind="ExternalInput")
    wzr_d = nc.dram_tensor("wzr", [128, KC * 2 * JT * 128], bf16, kind="ExternalInput")
    wht_d = nc.dram_tensor("wht", [128, KC * JT * 128], bf16, kind="ExternalInput")
    wlin_d = nc.dram_tensor("wlin", [128, KC * O], bf16, kind="ExternalInput")
    a0z_d = nc.dram_tensor("a0z", [128, FR], bf16, kind="ExternalInput")
    a0r_d = nc.dram_tensor("a0r", [128, FR], bf16, kind="ExternalInput")
    a0h_d = nc.dram_tensor("a0h", [128, FR], bf16, kind="ExternalInput")
    ident_d = nc.dram_tensor("ident", [128, 128], bf16, kind="ExternalInput")
    ones_d = nc.dram_tensor("ones64", [1, BL], bf16, kind="ExternalInput")
    blinr_d = nc.dram_tensor("blinr", [1, O], bf16, kind="ExternalInput")
    out_d = nc.dram_tensor("out", [T, BL, O], f32, kind="ExternalOutput")

    with tile.TileContext(nc) as tc, ExitStack() as ctx:
        constp = ctx.enter_context(tc.tile_pool(name="const", bufs=1))
        gpool = ctx.enter_context(tc.tile_pool(name="gam", bufs=2))
        statep = ctx.enter_context(tc.tile_pool(name="state", bufs=1))
        hdp = ctx.enter_context(tc.tile_pool(name="hd", bufs=2))
        actp = ctx.enter_context(tc.tile_pool(name="act", bufs=2))
        pzp = ctx.enter_context(tc.tile_pool(name="pz", bufs=1, space="PSUM"))
        prp = ctx.enter_context(tc.tile_pool(name="pr", bufs=1, space="PSUM"))
        php0 = ctx.enter_context(tc.tile_pool(name="ph0", bufs=1, space="PSUM"))
        php1 = ctx.enter_context(tc.tile_pool(name="ph1", bufs=1, space="PSUM"))
        pjp = ctx.enter_context(tc.tile_pool(name="pj", bufs=2, space="PSUM"))

        wzr = constp.tile([128, KC * 2 * JT * 128], bf16)
        nc.sync.dma_start(wzr[:], wzr_d[:])
        wht = constp.tile([128, KC * JT * 128], bf16)
        nc.sync.dma_start(wht[:], wht_d[:])
        wlin = constp.tile([128, KC * O], bf16)
        nc.sync.dma_start(wlin[:], wlin_d[:])
        a0z = constp.tile([128, FR], bf16)
        nc.sync.dma_start(a0z[:], a0z_d[:])
        a0r = constp.tile([128, FR], bf16)
        nc.sync.dma_start(a0r[:], a0r_d[:])
        a0h = constp.tile([128, FR], bf16)
        nc.sync.dma_start(a0h[:], a0h_d[:])
        ident = constp.tile([128, 128], bf16)
        nc.sync.dma_start(ident[:], ident_d[:])
        ones64 = constp.tile([1, BL], bf16)
        nc.sync.dma_start(ones64[:], ones_d[:])
        blinr = constp.tile([1, O], bf16)
        nc.sync.dma_start(blinr[:], blinr_d[:])

        h = statep.tile([128, FR], f32)
        nc.vector.memset(h[:], 0.0)

        def wzr_blk(g, jo, kc):
            i = ((kc * 2 + g) * JT + jo) * 128
            return wzr[:, i:i + 128]

        def wht_blk(jo, kc):
            i = (kc * JT + jo) * 128
            return wht[:, i:i + 128]

        # gamma chunks, preloaded half a chunk ahead
        chunks = {}

        def ensure_chunk(c):
            if c in chunks or c * GCH >= T:
                return
            t0 = c * GCH
            t1 = min(t0 + GCH, T)
            gt = gpool.tile([128, GCH * FR], f32, tag="gchunk")
            nc.sync.dma_start(gt[:, 0:(t1 - t0) * FR], gam_d[:, t0:t1, :])
            chunks[c] = gt

        def gamma_half(tt, hf):
            c2, o2 = divmod(tt, GCH)
            return chunks[c2][:, o2 * FR + hf * HB: o2 * FR + (hf + 1) * HB]

        ensure_chunk(0)

        # step-0 decayed state is zero
        hdf = hdp.tile([128, FR], f32, tag="hdf")
        nc.vector.memset(hdf[:], 0.0)
        hdb = hdp.tile([128, FR], bf16, tag="hdb")
        nc.vector.memset(hdb[:], 0.0)

        hbf_prev = None
        pj_prev = None

        for t in range(T):
            c, o = divmod(t, GCH)
            if o == GCH // 2:
                ensure_chunk(c + 1)

            # ---- output DMA for step t-1 (projection ran at the end of t-1)
            if pj_prev is not None:
                osb = actp.tile([BL, O], f32, tag="osb")
                nc.scalar.copy(osb[:], pj_prev[:])
                nc.sync.dma_start(out_d[t - 1], osb[:])

            # ---- r gate matmuls, jo-major: each pr j-slice completes after 4
            # MMs so sigmoid(r) halves start while later slices still run
            pr = prp.tile([128, FR], f32, tag="pr")
            nc.tensor.matmul(pr[:], ident[:], a0r[:], start=True, stop=False)
            for jo in range(JT):
                for kc in range(KC):
                    nc.tensor.matmul(
                        pr[:, jo * BL:(jo + 1) * BL],
                        wzr_blk(1, jo, kc),
                        hdb[:, kc * BL:(kc + 1) * BL],
                        start=False, stop=(kc == KC - 1),
                    )
            rb = actp.tile([128, FR], bf16, tag="rb")
            nc.scalar.activation(rb[:, 0:HB], pr[:, 0:HB], AF.Sigmoid)
            nc.scalar.activation(rb[:, HB:FR], pr[:, HB:FR], AF.Sigmoid)
            rh = hdp.tile([128, FR], bf16, tag="rh")
            nc.vector.tensor_mul(rh[:, 0:HB], rb[:, 0:HB], hdb[:, 0:HB])
            nc.vector.tensor_mul(rh[:, HB:FR], rb[:, HB:FR], hdb[:, HB:FR])

            # ---- z gate first half (jo 0,1)
            pz = pzp.tile([128, FR], f32, tag="pz")
            nc.tensor.matmul(pz[:], ident[:], a0z[:], start=True, stop=False)
            for jo in (0, 1):
                for kc in range(KC):
                    nc.tensor.matmul(
                        pz[:, jo * BL:(jo + 1) * BL],
                        wzr_blk(0, jo, kc),
                        hdb[:, kc * BL:(kc + 1) * BL],
                        start=False, stop=(kc == KC - 1),
                    )

            # ---- candidate gate, kc-chunks 0,1 (gated only by rh half 0)
            ph0 = php0.tile([128, HB], f32, tag="ph0")
            ph1 = php1.tile([128, HB], f32, tag="ph1")
            nc.tensor.matmul(ph0[:], ident[:], a0h[:, 0:HB], start=True, stop=False)
            nc.tensor.matmul(ph1[:], ident[:], a0h[:, HB:FR], start=True, stop=False)
            for kc in (0, 1):
                for jo in range(JT):
                    tgt = ph0 if jo < 2 else ph1
                    nc.tensor.matmul(
                        tgt[:, (jo % 2) * BL:(jo % 2 + 1) * BL],
                        wht_blk(jo, kc),
                        rh[:, kc * BL:(kc + 1) * BL],
                        start=False, stop=False,
                    )

            # ---- z gate second half (jo 2,3)
            for jo in (2, 3):
                for kc in range(KC):
                    nc.tensor.matmul(
                        pz[:, jo * BL:(jo + 1) * BL],
                        wzr_blk(0, jo, kc),
                        hdb[:, kc * BL:(kc + 1) * BL],
                        start=False, stop=(kc == KC - 1),
                    )
            zf = actp.tile([128, FR], f32, tag="zf")
            nc.scalar.activation(zf[:, 0:HB], pz[:, 0:HB], AF.Sigmoid)
            nc.scalar.activation(zf[:, HB:FR], pz[:, HB:FR], AF.Sigmoid)

            # ---- candidate gate, kc-chunks 2,3; jo 0,1 slices finish first so
            # tanh(half 0) can start while jo 2,3 still accumulate
            for jo in (0, 1):
                nc.tensor.matmul(
                    ph0[:, jo * BL:(jo + 1) * BL], wht_blk(jo, 2),
                    rh[:, 2 * BL:3 * BL], start=False, stop=False)
                nc.tensor.matmul(
                    ph0[:, jo * BL:(jo + 1) * BL], wht_blk(jo, 3),
                    rh[:, 3 * BL:4 * BL], start=False, stop=True)
            for jo in (2, 3):
                nc.tensor.matmul(
                    ph1[:, (jo - 2) * BL:(jo - 1) * BL], wht_blk(jo, 2),
                    rh[:, 2 * BL:3 * BL], start=False, stop=False)
                nc.tensor.matmul(
                    ph1[:, (jo - 2) * BL:(jo - 1) * BL], wht_blk(jo, 3),
                    rh[:, 3 * BL:4 * BL], start=False, stop=True)

            # ---- blend: h = (1-z)*hdec + z*htl, with (1-z)*hdec computed
            # off the tanh critical path
            zm = actp.tile([128, FR], f32, tag="zm")
            nc.vector.tensor_scalar(zm[:, 0:HB], zf[:, 0:HB], -1.0, 1.0,
                                    mybir.AluOpType.mult, mybir.AluOpType.add)
            pp_ = actp.tile([128, FR], f32, tag="pp")
            nc.vector.tensor_mul(pp_[:, 0:HB], zm[:, 0:HB], hdf[:, 0:HB])
            nc.vector.tensor_scalar(zm[:, HB:FR], zf[:, HB:FR], -1.0, 1.0,
                                    mybir.AluOpType.mult, mybir.AluOpType.add)
            nc.vector.tensor_mul(pp_[:, HB:FR], zm[:, HB:FR], hdf[:, HB:FR])

            hdf_n = hdb_n = None
            if t + 1 < T:
                hdf_n = hdp.tile([128, FR], f32, tag="hdf")
                hdb_n = hdp.tile([128, FR], bf16, tag="hdb")
            for hf, ph in ((0, ph0), (1, ph1)):
                sl = slice(hf * HB, (hf + 1) * HB)
                htl = actp.tile([128, HB], f32, tag=f"htl{hf}")
                nc.scalar.activation(htl[:], ph[:], AF.Tanh)
                qq = actp.tile([128, HB], f32, tag=f"qq{hf}")
                nc.vector.tensor_mul(qq[:], zf[:, sl], htl[:])
                nc.vector.tensor_add(h[:, sl], qq[:], pp_[:, sl])
                if t + 1 < T:
                    # bf16 decayed state straight from the fp32 mul (cast on write)
                    nc.vector.tensor_mul(hdb_n[:, sl], gamma_half(t + 1, hf), h[:, sl])
            if t + 1 < T:
                nc.vector.tensor_mul(hdf_n[:], chunks[(t + 1) // GCH][
                    :, ((t + 1) % GCH) * FR:((t + 1) % GCH + 1) * FR], h[:])
                hdf, hdb = hdf_n, hdb_n

            hbf = actp.tile([128, FR], bf16, tag="hbf")
            nc.scalar.copy(hbf[:], h[:])
            hbf_prev = hbf

            # ---- projection of h(t) at the end of the PE stream (fills the
            # tanh/blend tail); DMA'd out at the start of step t+1
            pj_prev = pjp.tile([BL, O], f32, tag="pj")
            nc.tensor.matmul(pj_prev[:], ones64[:], blinr[:], start=True, stop=False)
            for kc in range(KC):
                nc.tensor.matmul(
                    pj_prev[:],
                    hbf_prev[:, kc * BL:(kc + 1) * BL],
                    wlin[:, kc * O:(kc + 1) * O],
                    start=False, stop=(kc == KC - 1),
                )

        osb = actp.tile([BL, O], f32, tag="osb")
        nc.scalar.copy(osb[:], pj_prev[:])
        nc.sync.dma_start(out_d[T - 1], osb[:])

    nc.compile()
    _BUILD_CACHE["nc"] = nc
    return nc


def _host_prep(C, t, Wz, bz, Wr, br, Wh, bh, Wgh, bgh, Wlin, blin):
    """Build per-core input maps (all the precomputed, packed device tensors)."""
    bf = ml_dtypes.bfloat16

    s = Wgh.sum(axis=0)  # (H,)
    t3 = t[:, :, 0]  # (T,B)
    dt = np.concatenate([np.zeros((1, B), np.float32), t3[1:] - t3[:-1]], axis=0)
    # gamma (T,B,H) fp32
    gam = np.exp(-np.maximum(dt[:, :, None] * s[None, None, :] + bgh[None, None, :], 0.0)).astype(np.float32)

    def gate_const(W, b):
        # C @ W_x + colsum(W_m) + b  -> (B,H)
        return C @ W[0:H] + (W[2 * H:3 * H].sum(axis=0) + b)[None, :]

    Az0 = gate_const(Wz, bz).astype(np.float32)
    Ar0 = gate_const(Wr, br).astype(np.float32)
    Ah0 = gate_const(Wh, bh).astype(np.float32)

    Wg = np.stack([Wz[H:2 * H], Wr[H:2 * H]])  # (2,H,H)
    # wzr packed: [k, (kc,g,jo,m)]
    wzr = Wg.reshape(2, KC, 128, JT, 128).transpose(2, 1, 0, 3, 4).reshape(128, KC * 2 * JT * 128)
    wht = Wh[H:2 * H].reshape(KC, 128, JT, 128).transpose(1, 0, 2, 3).reshape(128, KC * JT * 128)
    wlin = Wlin.reshape(KC, 128, O).transpose(1, 0, 2).reshape(128, KC * O)
    wzr = np.ascontiguousarray(wzr, dtype=bf)
    wht = np.ascontiguousarray(wht, dtype=bf)
    wlin = np.ascontiguousarray(wlin, dtype=bf)
    ident = np.eye(128, dtype=bf)

    in_maps = []
    for i in range(NCORES):
        sl = slice(i * BL, (i + 1) * BL)
        gf = gam[:, sl, :]  # (T,BL,H)
        # gam packed: [p, t, kt*BL+b]
        gp = np.ascontiguousarray(gf.reshape(T, BL, KC, 128).transpose(3, 0, 2, 1).reshape(128, T, KC * BL))

        def packA(A):
            return np.ascontiguousarray(
                A[sl].reshape(BL, JT, 128).transpose(2, 1, 0).reshape(128, JT * BL), dtype=bf)

        in_maps.append({
            "gam": gp,
            "wzr": wzr,
            "wht": wht,
            "wlin": wlin,
            "a0z": packA(Az0),
            "a0r": packA(Ar0),
            "a0h": packA(Ah0),
            "ident": ident,
            "ones64": np.ones((1, BL), dtype=bf),
            "blinr": np.ascontiguousarray(blin.reshape(1, O), dtype=bf),
        })
    return in_maps


def kernel(C, t, mask, Wz, bz, Wr, br, Wh, bh, Wgh, bgh, wgx, bgx, Wlin, blin,
           _trace=False, _trace_kwargs=None):
    C = np.asarray(C, np.float32)
    t = np.asarray(t, np.float32)
    nc = _build_program()
    in_maps = _host_prep(C, t,
                         np.asarray(Wz, np.float32), np.asarray(bz, np.float32),
                         np.asarray(Wr, np.float32), np.asarray(br, np.float32),
                         np.asarray(Wh, np.float32), np.asarray(bh, np.float32),
                         np.asarray(Wgh, np.float32), np.asarray(bgh, np.float32),
                         np.asarray(Wlin, np.float32), np.asarray(blin, np.float32))

    from concourse.bass_utils import run_bass_kernel_spmd
    res = run_bass_kernel_spmd(nc, in_maps, list(range(NCORES)),
                               trace=_trace, **(_trace_kwargs or {}))
    outs = [res.results[i]["out"] for i in range(NCORES)]
    full = np.concatenate(outs, axis=1).astype(np.float32)  # (T,B,O)
    kernel._last_results = res
    return full



# revision 3
# speedup vs baseline: 1.4217x; 1.4217x over previous
"""GRU-D decoder kernel for Trainium2 (8 NeuronCores, data-parallel over batch).

Math (mask == ones everywhere, which the reference hardcodes):
  x_hat = C (constant), d = dt broadcast, gamma_x unused.
  gamma[t,b,j] = exp(-relu(dt[t,b] * colsum(Wgh)[j] + bgh[j]))   (precomputed host-side)
  per step: hd = gamma_t * h
            z = sigmoid(hd @ Wz_h + Az0);  r = sigmoid(hd @ Wr_h + Ar0)
            htl = tanh((r*hd) @ Wh_h + Ah0)
            h = hd + z*(htl - hd)
  out[t] = h_t @ Wlin + blin
  where A?0 = C @ W?_x + colsum(W?_m) + b?  (time-constant, precomputed host-side).

Device design (v2):
  - Transposed world: H folded onto 128 partitions (4 chunks of 128), batch=64
    on the free axis.  Weight-stationary (form 2) gate matmuls, N=64 streams.
  - Gate weights quantized to fp8e4 * 256 (stationary; FWL -> 27ns LDW), moving
    operand hd stays bf16; the 1/256 is folded into the activation scale.
  - z and r share one PSUM bank [128, 512]; a single ident-matmul streams the
    (pre-scaled) gate constants into PSUM to start both accumulation groups.
  - State update runs on DVE with gamma folded in algebraically:
      hd' = gamma'*hd + (gamma'*z)*(htl - hd)
    where e1 = gamma'*hd (GPSIMD, off-path), e2 = gamma'*z (DVE, off-path),
    leaving only d1 = htl-hd, f = e2*d1, hd' = e1+f on the critical path.
  - h_t = hd + z*d1 (GPSIMD, slack-tolerant) lands in a big hs history tile;
    the output projection (form 2, bf16 Wlin) runs as a rolling 4-step-batched
    matmul that doubles as a PE gap filler; +blin and the bf16 cast are fused
    into the DVE PSUM-evacuation op.  Output DMA'd as bf16, reassembled on host.
"""

import numpy as np
import ml_dtypes

T, B, H, O = 100, 512, 512, 512
NCORES = 8
BL = B // NCORES  # 64
KC = 4            # contraction chunks of 128
JT = 4            # output j-tiles of 128
FR = JT * BL      # 256
GCH = 10          # gamma chunk (steps per DMA)
PJ = 4            # projection flush period (steps)
WSCALE = 256.0    # fp8 gate-weight scale (undone in the activation)

_BUILD_CACHE = {}


def _build_program():
    if "nc" in _BUILD_CACHE:
        return _BUILD_CACHE["nc"]

    import concourse.tile as tile
    import concourse.mybir as mybir
    from concourse import bacc
    from contextlib import ExitStack

    f32 = mybir.dt.float32
    bf16 = mybir.dt.bfloat16
    f8 = mybir.dt.float8e4
    AF = mybir.ActivationFunctionType
    ALU = mybir.AluOpType

    nc = bacc.Bacc("TRN2", target_bir_lowering=False, debug=False,
                   num_devices=NCORES)

    gam_d = nc.dram_tensor("gam", [128, T, FR], bf16, kind="ExternalInput")
    wg8_d = nc.dram_tensor("wg8", [128, 3 * KC * JT * 128], f8, kind="ExternalInput")
    wlin_d = nc.dram_tensor("wlin", [128, KC * JT * 128], bf16, kind="ExternalInput")
    a0zr_d = nc.dram_tensor("a0zr", [128, 2 * FR], bf16, kind="ExternalInput")
    a0h_d = nc.dram_tensor("a0h", [128, FR], bf16, kind="ExternalInput")
    ident_d = nc.dram_tensor("ident", [128, 128], f8, kind="ExternalInput")
    blin_d = nc.dram_tensor("blin", [128, JT], f32, kind="ExternalInput")
    out_d = nc.dram_tensor("out", [JT, 128, T, BL], bf16, kind="ExternalOutput")

    with tile.TileContext(nc) as tc, ExitStack() as ctx:
        constp = ctx.enter_context(tc.tile_pool(name="const", bufs=1))
        bigp = ctx.enter_context(tc.tile_pool(name="big", bufs=1))
        statep = ctx.enter_context(tc.tile_pool(name="state", bufs=2))
        workp = ctx.enter_context(tc.tile_pool(name="work", bufs=2))
        stagep = ctx.enter_context(tc.tile_pool(name="stage", bufs=3))
        pzrp = ctx.enter_context(tc.tile_pool(name="pzr", bufs=2, space="PSUM"))
        phtp = ctx.enter_context(tc.tile_pool(name="pht", bufs=2, space="PSUM"))
        ppjp = ctx.enter_context(tc.tile_pool(name="ppj", bufs=1, space="PSUM"))

        wg8 = constp.tile([128, 3 * KC * JT * 128], f8)
        nc.sync.dma_start(wg8[:], wg8_d[:])
        wlin = constp.tile([128, KC * JT * 128], bf16)
        nc.sync.dma_start(wlin[:], wlin_d[:])
        a0zr = constp.tile([128, 2 * FR], bf16)
        nc.sync.dma_start(a0zr[:], a0zr_d[:])
        a0h = constp.tile([128, FR], bf16)
        nc.sync.dma_start(a0h[:], a0h_d[:])
        ident = constp.tile([128, 128], f8)
        nc.sync.dma_start(ident[:], ident_d[:])
        blin = constp.tile([128, JT], f32)
        nc.sync.dma_start(blin[:], blin_d[:])

        # gamma history + h history as big 3D tiles; chunked DMA into slices
        gam = bigp.tile([128, T, FR], bf16)
        hs = bigp.tile([128, T, FR], bf16)

        def gam_fetch(c):
            t0 = c * GCH
            t1 = min(t0 + GCH, T)
            if t0 < T:
                nc.sync.dma_start(gam[:, t0:t1, :], gam_d[:, t0:t1, :])

        gam_fetch(0)
        gam_fetch(1)

        def wg_blk(g, kc, jo):
            i = ((g * KC + kc) * JT + jo) * 128
            return wg8[:, i:i + 128]

        def wl_blk(kc, m):
            i = (kc * JT + m) * 128
            return wlin[:, i:i + 128]

        hdb = statep.tile([128, FR], bf16, tag="hdb")
        nc.vector.memset(hdb[:], 0.0)

        def emit_proj(m, base, nts):
            """out^T[m] for steps [base, base+nts): 4 accumulating MMs N=nts*64,
            then fused (+blin, ->bf16) DVE evacuation and the output DMA."""
            pj = ppjp.tile([128, PJ * BL], f32, tag=f"pj{m}")
            for kc in range(KC):
                nc.tensor.matmul(
                    pj[:, 0:nts * BL],
                    wl_blk(kc, m),
                    hs[:, base:base + nts, kc * BL:(kc + 1) * BL],
                    start=(kc == 0), stop=(kc == KC - 1),
                )
            stg = stagep.tile([128, PJ * BL], bf16, tag="stg")
            nc.vector.tensor_scalar(stg[:, 0:nts * BL], pj[:, 0:nts * BL],
                                    blin[:, m:m + 1], None, ALU.add)
            nc.sync.dma_start(out_d[m][:, base:base + nts, :], stg[:, 0:nts * BL])

        for t in range(T):
            c, o = divmod(t, GCH)
            if o == 0 and t > 0:
                gam_fetch(c + 1)
            last = t + 1 >= T

            if not last:
                gn = gam[:, t + 1, :]
                # e1 = gamma' * hd   (GPSIMD, ready at step start, off-path)
                e1 = workp.tile([128, FR], bf16, tag="e1")
                nc.gpsimd.tensor_mul(e1[:], gn, hdb[:])

            # ---- PE: bias inits (one ident LDW), r MMs, z MMs
            zr = pzrp.tile([128, 2 * FR], f32, tag="zr")
            nc.tensor.matmul(zr[:], ident[:], a0zr[:], start=True, stop=False)
            ht = phtp.tile([128, FR], f32, tag="ht")
            nc.tensor.matmul(ht[:], ident[:], a0h[:], start=True, stop=False)
            for jo in range(JT):
                for kc in range(KC):
                    nc.tensor.matmul(
                        zr[:, FR + jo * BL:FR + (jo + 1) * BL],
                        wg_blk(1, kc, jo), hdb[:, kc * BL:(kc + 1) * BL],
                        start=False, stop=False,
                    )
            for jo in range(JT):
                for kc in range(KC):
                    nc.tensor.matmul(
                        zr[:, jo * BL:(jo + 1) * BL],
                        wg_blk(0, kc, jo), hdb[:, kc * BL:(kc + 1) * BL],
                        start=False, stop=(jo == JT - 1 and kc == KC - 1),
                    )

            # ---- sigmoid(r) -> rh;  (scale undoes the fp8 weight scaling)
            rb = workp.tile([128, FR], bf16, tag="rb")
            nc.scalar.activation(rb[:], zr[:, FR:2 * FR], AF.Sigmoid,
                                 scale=1.0 / WSCALE)
            rh = workp.tile([128, FR], bf16, tag="rh")
            nc.vector.tensor_mul(rh[:], rb[:], hdb[:])

            # ---- rolling projection (PE gap filler) for steps [base, base+PJ)
            if t >= PJ:
                base = (t // PJ - 1) * PJ
                emit_proj(t % PJ, base, PJ)

            # ---- candidate MMs (wait on rh), k-outer
            for kc in range(KC):
                for jo in range(JT):
                    nc.tensor.matmul(
                        ht[:, jo * BL:(jo + 1) * BL],
                        wg_blk(2, kc, jo), rh[:, kc * BL:(kc + 1) * BL],
                        start=False, stop=(kc == KC - 1 and jo == JT - 1),
                    )

            # ---- sigmoid(z), e2 = gamma' * z (off critical path)
            zf = workp.tile([128, FR], bf16, tag="zf")
            nc.scalar.activation(zf[:], zr[:, 0:FR], AF.Sigmoid,
                                 scale=1.0 / WSCALE)
            if not last:
                e2 = workp.tile([128, FR], bf16, tag="e2")
                nc.vector.tensor_mul(e2[:], gn, zf[:])

            # ---- tanh -> tail
            htl = workp.tile([128, FR], bf16, tag="htl")
            nc.scalar.activation(htl[:], ht[:], AF.Tanh, scale=1.0 / WSCALE)
            d1 = workp.tile([128, FR], bf16, tag="d1")
            nc.vector.tensor_sub(d1[:], htl[:], hdb[:])
            if not last:
                f_ = workp.tile([128, FR], bf16, tag="f")
                nc.vector.tensor_mul(f_[:], e2[:], d1[:])
                hdb_n = statep.tile([128, FR], bf16, tag="hdb")
                nc.vector.tensor_add(hdb_n[:], e1[:], f_[:])

            # ---- h_t = hd + z*d1 into the history buffer (GPSIMD, slack ok)
            d2 = workp.tile([128, FR], bf16, tag="d2")
            nc.gpsimd.tensor_mul(d2[:], zf[:], d1[:])
            nc.gpsimd.tensor_add(hs[:, t, :], hdb[:], d2[:])

            if not last:
                hdb = hdb_n

        # ---- final projection flush for steps [T-PJ, T)
        for m in range(JT):
            emit_proj(m, T - PJ, PJ)

    nc.compile()

    # LDW dedup surgery: consecutive identical weight loads (same AP, no sems)
    # collapse to one -- the PE array already holds that stationary operand.
    deleted = 0
    for blk in nc.main_func.blocks:
        keep = []
        prev_key = None
        for ins in blk.instructions:
            nm = type(ins).__name__
            if nm == 'InstLdweights':
                key = str(ins.ins[0])
                has_sem = ins.sync_info is not None and (
                    len(ins.sync_info.on_wait) > 0 or len(ins.sync_info.on_update) > 0)
                if key == prev_key and not has_sem:
                    deleted += 1
                    continue
                prev_key = key
            elif nm == 'InstMatmult':
                pass
            elif nm in ('InstEventSemaphore', 'InstDrain'):
                pass
            else:
                prev_key = None
            keep.append(ins)
        blk.instructions[:] = keep

    _BUILD_CACHE["nc"] = nc
    return nc


def _host_prep(C, t, Wz, bz, Wr, br, Wh, bh, Wgh, bgh, Wlin, blin):
    bf = ml_dtypes.bfloat16
    f8 = ml_dtypes.float8_e4m3

    s = Wgh.sum(axis=0)  # (H,)
    t3 = t[:, :, 0]  # (T,B)
    dt = np.concatenate([np.zeros((1, B), np.float32), t3[1:] - t3[:-1]], axis=0)
    gam = np.exp(-np.maximum(
        dt[:, :, None] * s[None, None, :] + bgh[None, None, :], 0.0)).astype(np.float32)

    def gate_const(W, b):
        return C @ W[0:H] + (W[2 * H:3 * H].sum(axis=0) + b)[None, :]

    Az0 = gate_const(Wz, bz) * WSCALE
    Ar0 = gate_const(Wr, br) * WSCALE
    Ah0 = gate_const(Wh, bh) * WSCALE

    # gate weight tiles, fp8 * WSCALE, packed [p, ((g*KC+kc)*JT+jo)*128 + col]
    Wg = np.stack([Wz[H:2 * H], Wr[H:2 * H], Wh[H:2 * H]])  # (3,H,H)
    wg8 = (Wg * WSCALE).reshape(3, KC, 128, JT, 128).transpose(2, 0, 1, 3, 4)
    wg8 = np.ascontiguousarray(wg8.reshape(128, 3 * KC * JT * 128), dtype=f8)
    # wlin tiles bf16, packed [p, (kc*JT+m)*128 + col]
    wl = Wlin.reshape(KC, 128, JT, 128).transpose(1, 0, 2, 3)
    wl = np.ascontiguousarray(wl.reshape(128, KC * JT * 128), dtype=bf)
    identv = np.ascontiguousarray(np.eye(128), dtype=f8)
    blinT = np.ascontiguousarray(
        blin.reshape(JT, 128).T, dtype=np.float32)  # [128, JT]

    in_maps = []
    for i in range(NCORES):
        sl = slice(i * BL, (i + 1) * BL)
        gf = gam[:, sl, :]  # (T,BL,H)
        gp = np.ascontiguousarray(
            gf.reshape(T, BL, KC, 128).transpose(3, 0, 2, 1).reshape(128, T, KC * BL),
            dtype=bf)

        def packA(A):
            return A[sl].reshape(BL, JT, 128).transpose(2, 1, 0).reshape(128, JT * BL)

        a0zr = np.ascontiguousarray(
            np.concatenate([packA(Az0), packA(Ar0)], axis=1), dtype=bf)
        in_maps.append({
            "gam": gp,
            "wg8": wg8,
            "wlin": wl,
            "a0zr": a0zr,
            "a0h": np.ascontiguousarray(packA(Ah0), dtype=bf),
            "ident": identv,
            "blin": blinT,
        })
    return in_maps


def kernel(C, t, mask, Wz, bz, Wr, br, Wh, bh, Wgh, bgh, wgx, bgx, Wlin, blin,
           _trace=False, _trace_kwargs=None):
    C = np.asarray(C, np.float32)
    t = np.asarray(t, np.float32)
    nc = _build_program()
    in_maps = _host_prep(C, t,
                         np.asarray(Wz, np.float32), np.asarray(bz, np.float32),
                         np.asarray(Wr, np.float32), np.asarray(br, np.float32),
                         np.asarray(Wh, np.float32), np.asarray(bh, np.float32),
                         np.asarray(Wgh, np.float32), np.asarray(bgh, np.float32),
                         np.asarray(Wlin, np.float32), np.asarray(blin, np.float32))

    from concourse.bass_utils import run_bass_kernel_spmd
    res = run_bass_kernel_spmd(nc, in_maps, list(range(NCORES)),
                               trace=_trace, **(_trace_kwargs or {}))
    outs = []
    for i in range(NCORES):
        o4 = np.asarray(res.results[i]["out"], dtype=np.float32)  # (JT,128,T,BL)
        outs.append(o4.transpose(2, 3, 0, 1).reshape(T, BL, O))
    full = np.concatenate(outs, axis=1)  # (T,B,O)
    kernel._last_results = res
    return full


# revision 4
# speedup vs baseline: 1.4924x; 1.0497x over previous
"""GRU-D decoder kernel for Trainium2 (8 NeuronCores, data-parallel over batch).

Math (mask == ones everywhere, which the reference hardcodes):
  x_hat = C (constant), d = dt broadcast, gamma_x unused.
  gamma[t,b,j] = exp(-relu(dt[t,b] * colsum(Wgh)[j] + bgh[j]))   (precomputed host-side)
  per step: hd = gamma_t * h
            z = sigmoid(hd @ Wz_h + Az0);  r = sigmoid(hd @ Wr_h + Ar0)
            htl = tanh((r*hd) @ Wh_h + Ah0)
            h = hd + z*(htl - hd)
  out[t] = h_t @ Wlin + blin
  where A?0 = C @ W?_x + colsum(W?_m) + b?  (time-constant, precomputed host-side).

Device design (v3):
  - Transposed world: H folded onto 128 partitions (4 chunks of 128), batch=64
    on the free axis.  Weight-stationary (form 2) gate matmuls, N=64 streams.
  - Gate weights quantized to fp8e4 * 256 (stationary; fast weight load), moving
    operand hd stays bf16; the 1/256 is folded into the activation scale.
  - z and r share one PSUM bank [128, 512]; a single ident-matmul streams the
    (pre-scaled) gate constants into PSUM to start both accumulation groups.
  - Critical path holds only two DVE ops after tanh:
      hd' = gamma'*(hd + z*(htl-hd)) = u + e2*htl
    with e1 = gamma'*hd (GPSIMD, step start), e2 = gamma'*z, q = z*e1,
    u = e1 - q all off-path after sigmoid(z).  Activations and the update are
    processed in two 128-column halves; the next step's matmuls run k-outer so
    they start as soon as the first half of hd' lands.
  - h_t = hd + z*(htl-hd) is recomputed entirely on GPSIMD (slack-tolerant)
    into a big h-history tile; the output projection (form 2, bf16 Wlin) runs
    as a rolling 4-step-batched matmul split around the candidate matmuls to
    fill PE stalls; +blin and the bf16 cast are fused into the DVE PSUM
    evacuation.  Output is DMA'd as bf16 and reassembled host-side.
"""

import numpy as np
import ml_dtypes

T, B, H, O = 100, 512, 512, 512
NCORES = 8
BL = B // NCORES  # 64
KC = 4            # contraction chunks of 128
JT = 4            # output j-tiles of 128
FR = JT * BL      # 256
HB = FR // 2      # 128 (half of the free dim)
GCH = 10          # gamma chunk (steps per DMA)
PJ = 4            # projection flush period (steps)
WSCALE = 256.0    # fp8 gate-weight scale (undone in the activation)

_BUILD_CACHE = {}


def _build_program():
    if "nc" in _BUILD_CACHE:
        return _BUILD_CACHE["nc"]

    import concourse.tile as tile
    import concourse.mybir as mybir
    from concourse import bacc
    from contextlib import ExitStack

    f32 = mybir.dt.float32
    bf16 = mybir.dt.bfloat16
    f8 = mybir.dt.float8e4
    AF = mybir.ActivationFunctionType
    ALU = mybir.AluOpType

    nc = bacc.Bacc("TRN2", target_bir_lowering=False, debug=False,
                   num_devices=NCORES)

    gam_d = nc.dram_tensor("gam", [128, T, FR], bf16, kind="ExternalInput")
    wg8_d = nc.dram_tensor("wg8", [128, 3 * KC * JT * 128], f8, kind="ExternalInput")
    wlin_d = nc.dram_tensor("wlin", [128, KC * JT * 128], bf16, kind="ExternalInput")
    a0zr_d = nc.dram_tensor("a0zr", [128, 2 * FR], bf16, kind="ExternalInput")
    a0h_d = nc.dram_tensor("a0h", [128, FR], bf16, kind="ExternalInput")
    ident_d = nc.dram_tensor("ident", [128, 128], f8, kind="ExternalInput")
    blin_d = nc.dram_tensor("blin", [128, JT], f32, kind="ExternalInput")
    out_d = nc.dram_tensor("out", [JT, 128, T, BL], bf16, kind="ExternalOutput")

    with tile.TileContext(nc) as tc, ExitStack() as ctx:
        constp = ctx.enter_context(tc.tile_pool(name="const", bufs=1))
        bigp = ctx.enter_context(tc.tile_pool(name="big", bufs=1))
        statep = ctx.enter_context(tc.tile_pool(name="state", bufs=2))
        workp = ctx.enter_context(tc.tile_pool(name="work", bufs=2))
        stagep = ctx.enter_context(tc.tile_pool(name="stage", bufs=3))
        pzrp = ctx.enter_context(tc.tile_pool(name="pzr", bufs=2, space="PSUM"))
        phtp = ctx.enter_context(tc.tile_pool(name="pht", bufs=2, space="PSUM"))
        ppjp = ctx.enter_context(tc.tile_pool(name="ppj", bufs=1, space="PSUM"))

        wg8 = constp.tile([128, 3 * KC * JT * 128], f8)
        nc.sync.dma_start(wg8[:], wg8_d[:])
        wlin = constp.tile([128, KC * JT * 128], bf16)
        nc.sync.dma_start(wlin[:], wlin_d[:])
        a0zr = constp.tile([128, 2 * FR], bf16)
        nc.sync.dma_start(a0zr[:], a0zr_d[:])
        a0h = constp.tile([128, FR], bf16)
        nc.sync.dma_start(a0h[:], a0h_d[:])
        ident = constp.tile([128, 128], f8)
        nc.sync.dma_start(ident[:], ident_d[:])
        blin = constp.tile([128, JT], f32)
        nc.sync.dma_start(blin[:], blin_d[:])

        # gamma history + h history as big 3D tiles; chunked DMA into slices
        gam = bigp.tile([128, T, FR], bf16)
        hs = bigp.tile([128, T, FR], bf16)

        def gam_fetch(c):
            t0 = c * GCH
            t1 = min(t0 + GCH, T)
            if t0 < T:
                nc.sync.dma_start(gam[:, t0:t1, :], gam_d[:, t0:t1, :])

        gam_fetch(0)
        gam_fetch(1)

        def wg_blk(g, kc, jo):
            i = ((g * KC + kc) * JT + jo) * 128
            return wg8[:, i:i + 128]

        def wl_blk(kc, m):
            i = (kc * JT + m) * 128
            return wlin[:, i:i + 128]

        hdb = statep.tile([128, FR], bf16, tag="hdb")
        nc.vector.memset(hdb[:], 0.0)

        def proj_mms(m, base, kcs, pj):
            for kc in kcs:
                nc.tensor.matmul(
                    pj[:, 0:PJ * BL],
                    wl_blk(kc, m),
                    hs[:, base:base + PJ, kc * BL:(kc + 1) * BL],
                    start=(kc == 0), stop=(kc == KC - 1),
                )

        def proj_evac(m, base, pj):
            stg = stagep.tile([128, PJ * BL], bf16, tag="stg")
            nc.vector.tensor_scalar(stg[:], pj[:], blin[:, m:m + 1], None, ALU.add)
            nc.sync.dma_start(out_d[m][:, base:base + PJ, :], stg[:])

        for t in range(T):
            c, o = divmod(t, GCH)
            if o == 0 and t > 0:
                gam_fetch(c + 1)
            last = t + 1 >= T

            if not last:
                gn = gam[:, t + 1, :]
                # e1 = gamma' * hd   (GPSIMD, ready at step start, off-path)
                e1 = workp.tile([128, FR], bf16, tag="e1")
                nc.gpsimd.tensor_mul(e1[:], gn, hdb[:])

            # ---- PE: bias inits (one ident LDW), r MMs, z MMs (k-outer)
            zr = pzrp.tile([128, 2 * FR], f32, tag="zr")
            nc.tensor.matmul(zr[:], ident[:], a0zr[:], start=True, stop=False)
            ht = phtp.tile([128, FR], f32, tag="ht")
            nc.tensor.matmul(ht[:], ident[:], a0h[:], start=True, stop=False)
            for kc in range(KC):
                for jo in range(JT):
                    nc.tensor.matmul(
                        zr[:, FR + jo * BL:FR + (jo + 1) * BL],
                        wg_blk(1, kc, jo), hdb[:, kc * BL:(kc + 1) * BL],
                        start=False, stop=False,
                    )
            for kc in range(KC):
                for jo in range(JT):
                    nc.tensor.matmul(
                        zr[:, jo * BL:(jo + 1) * BL],
                        wg_blk(0, kc, jo), hdb[:, kc * BL:(kc + 1) * BL],
                        start=False, stop=(jo == JT - 1 and kc == KC - 1),
                    )

            # ---- sigmoid(r) -> rh, in halves (scale undoes fp8 weight scale)
            rb = workp.tile([128, FR], bf16, tag="rb")
            rh = workp.tile([128, FR], bf16, tag="rh")
            for h0 in (0, HB):
                nc.scalar.activation(rb[:, h0:h0 + HB], zr[:, FR + h0:FR + h0 + HB],
                                     AF.Sigmoid, scale=1.0 / WSCALE)
                nc.vector.tensor_mul(rh[:, h0:h0 + HB], rb[:, h0:h0 + HB],
                                     hdb[:, h0:h0 + HB])

            # ---- rolling projection (PE gap filler) for steps [base, base+PJ)
            pj = pbase = None
            if t >= PJ:
                pbase = (t // PJ - 1) * PJ
                pj = ppjp.tile([128, PJ * BL], f32, tag=f"pj{t % PJ}")
                proj_mms(t % PJ, pbase, (0, 1), pj)

            # ---- candidate MMs (k-outer: kc chunk waits only on its rh half)
            for kc in range(KC):
                for jo in range(JT):
                    nc.tensor.matmul(
                        ht[:, jo * BL:(jo + 1) * BL],
                        wg_blk(2, kc, jo), rh[:, kc * BL:(kc + 1) * BL],
                        start=False, stop=(kc == KC - 1 and jo == JT - 1),
                    )

            if pj is not None:
                proj_mms(t % PJ, pbase, (2, 3), pj)

            # ---- sigmoid(z); off-path pre-products for the state update
            zf = workp.tile([128, FR], bf16, tag="zf")
            nc.scalar.activation(zf[:], zr[:, 0:FR], AF.Sigmoid, scale=1.0 / WSCALE)
            if not last:
                e2 = workp.tile([128, FR], bf16, tag="e2")
                q = workp.tile([128, FR], bf16, tag="q")
                u = workp.tile([128, FR], bf16, tag="u")
                nc.vector.tensor_mul(e2[:], gn, zf[:])
                nc.vector.tensor_mul(q[:], zf[:], e1[:])
                nc.vector.tensor_sub(u[:], e1[:], q[:])

            # ---- tanh -> hd' = u + e2*htl, in halves
            htl = workp.tile([128, FR], bf16, tag="htl")
            if not last:
                v = workp.tile([128, FR], bf16, tag="v")
                hdb_n = statep.tile([128, FR], bf16, tag="hdb")
            for h0 in (0, HB):
                sl = slice(h0, h0 + HB)
                nc.scalar.activation(htl[:, sl], ht[:, sl], AF.Tanh,
                                     scale=1.0 / WSCALE)
                if not last:
                    nc.vector.tensor_mul(v[:, sl], e2[:, sl], htl[:, sl])
                    nc.vector.tensor_add(hdb_n[:, sl], u[:, sl], v[:, sl])

            # ---- h_t = hd + z*(htl-hd) on GPSIMD (slack-tolerant) -> history
            d1 = workp.tile([128, FR], bf16, tag="d1")
            nc.gpsimd.tensor_sub(d1[:], htl[:], hdb[:])
            d2 = workp.tile([128, FR], bf16, tag="d2")
            nc.gpsimd.tensor_mul(d2[:], zf[:], d1[:])
            nc.gpsimd.tensor_add(hs[:, t, :], hdb[:], d2[:])

            if pj is not None:
                proj_evac(t % PJ, pbase, pj)

            if not last:
                hdb = hdb_n

        # ---- final projection flush for steps [T-PJ, T)
        for m in range(JT):
            pj = ppjp.tile([128, PJ * BL], f32, tag=f"pj{m}")
            proj_mms(m, T - PJ, range(KC), pj)
            proj_evac(m, T - PJ, pj)

    nc.compile()

    # LDW dedup surgery: consecutive identical weight loads (same AP, no sems)
    # collapse to one -- the PE array already holds that stationary operand.
    for blk in nc.main_func.blocks:
        keep = []
        prev_key = None
        for ins in blk.instructions:
            nm = type(ins).__name__
            if nm == 'InstLdweights':
                key = str(ins.ins[0])
                has_sem = ins.sync_info is not None and (
                    len(ins.sync_info.on_wait) > 0 or len(ins.sync_info.on_update) > 0)
                if key == prev_key and not has_sem:
                    continue
                prev_key = key
            elif nm in ('InstMatmult', 'InstEventSemaphore', 'InstDrain'):
                pass
            else:
                prev_key = None
            keep.append(ins)
        blk.instructions[:] = keep

    _BUILD_CACHE["nc"] = nc
    return nc


def _host_prep(C, t, Wz, bz, Wr, br, Wh, bh, Wgh, bgh, Wlin, blin):
    bf = ml_dtypes.bfloat16
    f8 = ml_dtypes.float8_e4m3

    s = Wgh.sum(axis=0)  # (H,)
    t3 = t[:, :, 0]  # (T,B)
    dt = np.concatenate([np.zeros((1, B), np.float32), t3[1:] - t3[:-1]], axis=0)
    gam = np.exp(-np.maximum(
        dt[:, :, None] * s[None, None, :] + bgh[None, None, :], 0.0)).astype(np.float32)

    def gate_const(W, b):
        return C @ W[0:H] + (W[2 * H:3 * H].sum(axis=0) + b)[None, :]

    Az0 = gate_const(Wz, bz) * WSCALE
    Ar0 = gate_const(Wr, br) * WSCALE
    Ah0 = gate_const(Wh, bh) * WSCALE

    # gate weight tiles, fp8 * WSCALE, packed [p, ((g*KC+kc)*JT+jo)*128 + col]
    Wg = np.stack([Wz[H:2 * H], Wr[H:2 * H], Wh[H:2 * H]])  # (3,H,H)
    wg8 = (Wg * WSCALE).reshape(3, KC, 128, JT, 128).transpose(2, 0, 1, 3, 4)
    wg8 = np.ascontiguousarray(wg8.reshape(128, 3 * KC * JT * 128), dtype=f8)
    # wlin tiles bf16, packed [p, (kc*JT+m)*128 + col]
    wl = Wlin.reshape(KC, 128, JT, 128).transpose(1, 0, 2, 3)
    wl = np.ascontiguousarray(wl.reshape(128, KC * JT * 128), dtype=bf)
    identv = np.ascontiguousarray(np.eye(128), dtype=f8)
    blinT = np.ascontiguousarray(
        blin.reshape(JT, 128).T, dtype=np.float32)  # [128, JT]

    in_maps = []
    for i in range(NCORES):
        sl = slice(i * BL, (i + 1) * BL)
        gf = gam[:, sl, :]  # (T,BL,H)
        gp = np.ascontiguousarray(
            gf.reshape(T, BL, KC, 128).transpose(3, 0, 2, 1).reshape(128, T, KC * BL),
            dtype=bf)

        def packA(A):
            return A[sl].reshape(BL, JT, 128).transpose(2, 1, 0).reshape(128, JT * BL)

        a0zr = np.ascontiguousarray(
            np.concatenate([packA(Az0), packA(Ar0)], axis=1), dtype=bf)
        in_maps.append({
            "gam": gp,
            "wg8": wg8,
            "wlin": wl,
            "a0zr": a0zr,
            "a0h": np.ascontiguousarray(packA(Ah0), dtype=bf),
            "ident": identv,
            "blin": blinT,
        })
    return in_maps


def kernel(C, t, mask, Wz, bz, Wr, br, Wh, bh, Wgh, bgh, wgx, bgx, Wlin, blin,
           _trace=False, _trace_kwargs=None):
    C = np.asarray(C, np.float32)
    t = np.asarray(t, np.float32)
    nc = _build_program()
    in_maps = _host_prep(C, t,
                         np.asarray(Wz, np.float32), np.asarray(bz, np.float32),
                         np.asarray(Wr, np.float32), np.asarray(br, np.float32),
                         np.asarray(Wh, np.float32), np.asarray(bh, np.float32),
                         np.asarray(Wgh, np.float32), np.asarray(bgh, np.float32),
                         np.asarray(Wlin, np.float32), np.asarray(blin, np.float32))

    from concourse.bass_utils import run_bass_kernel_spmd
    res = run_bass_kernel_spmd(nc, in_maps, list(range(NCORES)),
                               trace=_trace, **(_trace_kwargs or {}))
    outs = []
    for i in range(NCORES):
        o4 = np.asarray(res.results[i]["out"], dtype=np.float32)  # (JT,128,T,BL)
        outs.append(o4.transpose(2, 3, 0, 1).reshape(T, BL, O))
    full = np.concatenate(outs, axis=1)  # (T,B,O)
    kernel._last_results = res
    return full
